# revision 1
# baseline (speedup 1.0000x reference)
"""AutoCorrelationLoss Trainium2 kernel (8-core SPMD, data-parallel over batch).

Math: for each row x (length L=8192), with com = L - 128 = 8064 = 63*128:
  ac[k] = mean(x0c * (Y_k - mean(Y_k)))  where x0c = x[:com] - mean(x[:com])
Since sum(x0c) = 0, the mean(Y_k) term vanishes:
  com * ac[k] = c[k] = sum_j x0c[j] * x[j+k]
Decompose j = 128*t + p (t<63, p<128) and let T[t, f] = x[128t + f] (f<256),
Tc = T[:, :128] - mean(x[:com]).  Then with H = Tc.T @ T  ([128, 256]):
  c[k] = sum_j H[j, j+k]   (a diagonal/skew sum, k = 0..128)
r[k] = ac[k]/ac[0] = c[k]/c[0];  loss = mean_{b,k} |r_fake - r_real|.

Per core: 4 batch rows x {fake, real} = 8 row-tensors. One fp32 matmul
[63,128]x[63,256] per row-tensor, a diagonal-stride DMA de-skews H so the
129 skew-sums become column sums done by a ones-matmul.
"""

import os
import sys

sys.path.insert(0, "/opt/trn_rl_repo")

import numpy as np

import concourse.bacc as bacc
import concourse.bass as bass
import concourse.mybir as mybir
import concourse.tile as tile
from concourse.bass_utils import run_bass_kernel_spmd
from concourse.tile_rust import add_dep_helper

B, L = 32, 8192
NCOEF = 128            # lags 0..128 -> 129 values
COM = L - NCOEF        # 8064 = 63 * 128
NT = 63                # contraction chunks
HALO = 256             # halo width per chunk
NK = NCOEF + 1         # 129
N_CORES = 8
ROWS_PER_CORE = B // N_CORES      # 4 batch rows per core
RT = 2 * ROWS_PER_CORE            # 8 row-tensors (fake rows then real rows)

FP32 = mybir.dt.float32


def build_program(debug_taps=False, reps=1, stop_after="full"):
    # stop_after: "loads" | "matmul" | "hsb" | "hd" | "diag" | "reduce" | "full"
    STAGES = ["loads", "matmul", "hsb", "hd", "diag", "reduce", "full"]
    lvl = STAGES.index(stop_after)
    nc = bacc.Bacc(
        "TRN2",
        target_bir_lowering=False,
        debug=False,
        num_devices=N_CORES,
    )

    xin = nc.dram_tensor("xin", (RT, L), FP32, kind="ExternalInput")
    out = nc.dram_tensor("out", (ROWS_PER_CORE, 1), FP32, kind="ExternalOutput")
    if debug_taps:
        hdram = nc.dram_tensor("hdram", (RT, 128, HALO), FP32,
                               kind="ExternalOutput")
        rdram = nc.dram_tensor("rdram", (128, RT * NK), FP32,
                               kind="ExternalOutput")
        cdram = nc.dram_tensor("cdram", (ROWS_PER_CORE, 2 * NK), FP32,
                               kind="ExternalOutput")

    with tile.TileContext(nc) as tc:
        with (
            tc.tile_pool(name="tpool", bufs=3) as tpool,
            tc.tile_pool(name="wpool", bufs=3) as wpool,
            tc.tile_pool(name="spool", bufs=4) as spool,
            tc.tile_pool(name="hsb", bufs=3) as hsbpool,
            tc.tile_pool(name="persist", bufs=1) as persist,
            tc.tile_pool(name="hd", bufs=RT, space=bass.MemorySpace.DRAM) as hdpool,
            tc.tile_pool(name="hps", bufs=4, space=bass.MemorySpace.PSUM) as hps,
            tc.tile_pool(name="bps", bufs=2, space=bass.MemorySpace.PSUM) as bps,
            tc.tile_pool(name="cps", bufs=2, space=bass.MemorySpace.PSUM) as cps,
        ):
            ones63 = persist.tile([NT, NT], FP32)
            nc.vector.memset(ones63[:], 1.0)
            ones128 = persist.tile([128, 1], FP32)
            nc.vector.memset(ones128[:], 1.0)
            # de-skewed diagonals for all 8 row-tensors, side by side
            rbig = persist.tile([128, RT * NK], FP32)

            def emit_rep():
              diag_reads = []
              ka = None
              for rt in range(RT):
                # --- contiguous loads: XA[t] = x[128t : 128t+128] (t<64),
                #     XB[t] = x[128(t+1) : 128(t+2)] (t<63)
                xa = tpool.tile([NT + 1, 128], FP32, tag="xa")
                nc.sync.dma_start(
                    xa[:], bass.AP(xin, rt * L, [[128, NT + 1], [1, 128]]))
                xb = tpool.tile([NT, 128], FP32, tag="xb")
                nc.sync.dma_start(
                    xb[:], bass.AP(xin, rt * L + 128, [[128, NT], [1, 128]]))
                if lvl == 0:
                    ka = xa
                    continue

                # --- mean of x[0:com]
                rowsum = spool.tile([NT, 1], FP32, tag="rowsum")
                nc.vector.tensor_reduce(
                    rowsum[:], xa[0:NT, :], mybir.AxisListType.X,
                    mybir.AluOpType.add,
                )
                # broadcast total over 63 partitions: ones63.T @ rowsum
                bcast = bps.tile([NT, 1], FP32, tag="bcast")
                nc.tensor.matmul(bcast[:], ones63[:], rowsum[:],
                                 start=True, stop=True)
                m0 = spool.tile([NT, 1], FP32, tag="m0")
                nc.scalar.mul(m0[:], bcast[:], 1.0 / COM)

                # --- centered stationary operand
                tc_tile = wpool.tile([NT, 128], FP32, tag="w")
                nc.vector.tensor_scalar_sub(tc_tile[:], xa[0:NT, :], m0[:])

                # --- H = Tc.T @ [XA | XB]  -> PSUM [128, 256]
                h_ps = hps.tile([128, HALO], FP32, tag="h")
                nc.tensor.matmul(h_ps[:, 0:128], tc_tile[:], xa[0:NT, :],
                                 start=True, stop=True)
                nc.tensor.matmul(h_ps[:, 128:HALO], tc_tile[:], xb[:],
                                 start=True, stop=True)
                if lvl == 1:
                    kat = spool.tile([1, 1], FP32, tag="ka")
                    nc.scalar.copy(kat[:], h_ps[0:1, 0:1])
                    ka = kat
                    continue

                # --- PSUM -> SBUF (alternate engines to balance load)
                h_sb = hsbpool.tile([128, HALO], FP32, tag="hsb")
                if rt % 2 == 0:
                    nc.vector.tensor_copy(h_sb[:], h_ps[:])
                else:
                    nc.scalar.copy(h_sb[:], h_ps[:])
                if lvl == 2:
                    ka = h_sb
                    continue

                # --- de-skew via DRAM bounce: R[j, k] = H[j, j + k]
                # Custom (non-slice) APs are invisible to Tile's dependency
                # tracker, so the read edges are added explicitly below.
                hd = hdpool.tile([128, HALO], FP32, tag="hd")
                hd_w = nc.sync.dma_start(hd[:], h_sb[:])
                if lvl == 3:
                    ka = h_sb
                    continue
                diag = bass.AP(hd[:].tensor, 0, [[HALO + 1, 128], [1, NK]])
                d_r = nc.sync.dma_start(rbig[:, rt * NK:(rt + 1) * NK], diag)
                add_dep_helper(d_r.ins, hd_w.ins, reason="deskew reads hd")
                diag_reads.append(d_r)
                if debug_taps:
                    nc.sync.dma_start(hdram[rt], hd[:])

              if lvl == 4:
                  ka = rbig
              if lvl < 5:
                  nc.sync.dma_start(out[0:1, 0:1], ka[0:1, 0:1])
                  return

              # --- column sums of rbig -> c-vectors, 3 matmuls of N=344
              csums = spool.tile([1, RT * NK], FP32, tag="csums")
              nchunk = RT * NK // 3        # 344
              cs_copies = []
              for i in range(3):
                  cs_ps = cps.tile([1, nchunk], FP32, tag="cs")
                  mm = nc.tensor.matmul(
                      cs_ps[:], ones128[:],
                      rbig[:, i * nchunk:(i + 1) * nchunk],
                      start=True, stop=True,
                  )
                  for d_r in diag_reads:
                      add_dep_helper(mm.ins, d_r.ins, reason="rbig ready")
                  cs_copies.append(
                      nc.scalar.copy(csums[:, i * nchunk:(i + 1) * nchunk],
                                     cs_ps[:]))

              if lvl < 6:
                  nc.sync.dma_start(out[0:1, 0:1], csums[0:1, 0:1])
                  return

              # --- scatter to [4, 2*129]: row b = [c_fake_b | c_real_b].
              # SBUF APs are partition-addressed, so the flat gather must go
              # through DRAM (flat byte addressing).
              cs_d = hdpool.tile([RT * NK], FP32, tag="csd")
              csd_w = nc.sync.dma_start(cs_d[:], csums[:])
              cs_mat = spool.tile([ROWS_PER_CORE, 2 * NK], FP32, tag="csmat")
              csrc = bass.AP(
                  cs_d[:].tensor, 0,
                  [[NK, ROWS_PER_CORE], [ROWS_PER_CORE * NK, 2], [1, NK]],
              )
              rearr = nc.sync.dma_start(cs_mat[:], csrc)
              add_dep_helper(rearr.ins, csd_w.ins, reason="cs_d ready")

              # --- normalize r = c / c0 (each half by its own c0)
              recf = spool.tile([ROWS_PER_CORE, 1], FP32, tag="recf")
              nc.vector.reciprocal(recf[:], cs_mat[:, 0:1])
              recr = spool.tile([ROWS_PER_CORE, 1], FP32, tag="recr")
              nc.vector.reciprocal(recr[:], cs_mat[:, NK:NK + 1])
              rf = spool.tile([ROWS_PER_CORE, NK], FP32, tag="rf")
              nc.vector.tensor_scalar_mul(rf[:], cs_mat[:, 0:NK], recf[:])
              rr = spool.tile([ROWS_PER_CORE, NK], FP32, tag="rr")
              nc.vector.tensor_scalar_mul(rr[:], cs_mat[:, NK:2 * NK], recr[:])

              # --- |r_fake - r_real| summed over k, per batch row
              diff = spool.tile([ROWS_PER_CORE, NK], FP32, tag="diff")
              nc.vector.tensor_sub(diff[:], rf[:], rr[:])
              absum = spool.tile([ROWS_PER_CORE, 1], FP32, tag="absum")
              nc.vector.tensor_reduce(
                  absum[:], diff[:], mybir.AxisListType.X, mybir.AluOpType.add,
                  apply_absolute_value=True,
              )
              nc.sync.dma_start(out[:], absum[:])
              if debug_taps:
                  nc.sync.dma_start(rdram[:], rbig[:])
                  nc.sync.dma_start(cdram[:], cs_mat[:])

            for _rep in range(reps):
                emit_rep()

    nc.compile()
    return nc


_CACHE = {}


def _get_program():
    if "nc" not in _CACHE:
        _CACHE["nc"] = build_program()
    return _CACHE["nc"]


def make_in_maps(fake: np.ndarray, real: np.ndarray):
    fake = np.asarray(fake, dtype=np.float32).reshape(B, L)
    real = np.asarray(real, dtype=np.float32).reshape(B, L)
    in_maps = []
    for c in range(N_CORES):
        rows = slice(c * ROWS_PER_CORE, (c + 1) * ROWS_PER_CORE)
        xin = np.concatenate([fake[rows], real[rows]], axis=0)
        in_maps.append({"xin": np.ascontiguousarray(xin)})
    return in_maps


def run(in_maps, **kwargs):
    """Run the SPMD program; returns (loss, BassKernelResults)."""
    res = run_bass_kernel_spmd(
        _get_program(), in_maps, list(range(N_CORES)), **kwargs
    )
    total = np.float64(0.0)
    for c in range(N_CORES):
        total += np.asarray(res.results[c]["out"], dtype=np.float64).sum()
    return np.float32(total / (B * NK)), res


def kernel(fake: np.ndarray, real: np.ndarray) -> np.ndarray:
    loss, _ = run(make_in_maps(fake, real))
    return loss



# revision 3
# speedup vs baseline: 1.4648x; 1.4648x over previous
"""AutoCorrelationLoss Trainium2 kernel (8-core SPMD, data-parallel over batch).

Math: for each row x (length L=8192), with com = L - 128 = 8064 = 63*128:
  ac[k] = mean(x0c * (Y_k - mean(Y_k)))  where x0c = x[:com] - mean(x[:com])
Since sum(x0c) = 0, the mean(Y_k) term vanishes:
  com * ac[k] = c[k] = sum_j x0c[j] * x[j+k]
Decompose j = 128*t + p (t<63, p<128), T[t, f] = x[128t + f] (f<256),
Tc = T[:, :128] - mean(x[:com]).  With H = Tc.T @ T  ([128, 256]):
  c[k] = sum_j H[j, j+k]   (a skew-diagonal sum, k = 0..128)
r[k] = c[k]/c[0]; loss = mean_{b,k} |r_fake - r_real|.  r[0] == 1 for both
sides, so the k=0 term contributes 0 — only c[1..128] and the divisor c[0]
are needed.

Per core: 4 batch rows x {fake, real} = 8 row-tensors (rt).  Pipeline:
  1. one DMA loads all 8 rows as [63, 8, 256] halo windows (fp32)
  2. batched stats: one 3D reduce + one ones-matmul broadcast -> means
  3. per-rt centering (fp32 -> bf16 weights); raw stream cast to bf16
  4. 8 bf16 matmuls [63,128]x[63,256] -> H_all [128, 8*256] (PSUM->SBUF bf16)
  5. deskew via DRAM bounce in bf16, split in halves to pipeline:
     write H_all -> hd, strided re-read R[j, rt, k] = H[j, rt*256+j+k]
  6. per-rt transposed matmul (lhsT = R[:, rt, 1:129], rhs = ones) gives
     c[1..128] as a [128, 8] column block; one more ones-matmul gives c0 [1,8]
  7. reciprocal of c0, broadcast to 128 partitions via a rank-1 matmul,
     r = c * (1/c0), loss partials = reduce |r_f - r_r| -> out [128, 1]
Host sums 8 cores' [128] partials and divides by B*(NCOEF+1).
"""

import os
import sys

sys.path.insert(0, "/opt/trn_rl_repo")

import numpy as np

import concourse.bacc as bacc
import concourse.bass as bass
import concourse.mybir as mybir
import concourse.tile as tile
from concourse.bass_utils import run_bass_kernel_spmd
from concourse.tile_rust import add_dep_helper

B, L = 32, 8192
NCOEF = 128            # lags 0..128 -> 129 values
COM = L - NCOEF        # 8064 = 63 * 128
NT = 63                # contraction chunks
HALO = 256             # halo width per chunk
NK = NCOEF + 1         # 129
N_CORES = 8
ROWS_PER_CORE = B // N_CORES      # 4 batch rows per core
RT = 2 * ROWS_PER_CORE            # 8 row-tensors (fake rows then real rows)

FP32 = mybir.dt.float32
BF16 = mybir.dt.bfloat16


def build_program():
    nc = bacc.Bacc(
        "TRN2",
        target_bir_lowering=False,
        debug=False,
        num_devices=N_CORES,
    )

    xin = nc.dram_tensor("xin", (RT, L), FP32, kind="ExternalInput")
    out = nc.dram_tensor("out", (128, 1), FP32, kind="ExternalOutput")

    with tile.TileContext(nc) as tc:
        with (
            tc.tile_pool(name="persist", bufs=1) as persist,
            tc.tile_pool(name="big", bufs=1) as bigp,
            tc.tile_pool(name="spool", bufs=4) as spool,
            tc.tile_pool(name="hdp", bufs=1, space=bass.MemorySpace.DRAM) as hdp,
            tc.tile_pool(name="hps", bufs=4, space=bass.MemorySpace.PSUM) as hps,
            tc.tile_pool(name="sps", bufs=1, space=bass.MemorySpace.PSUM) as sps,
        ):
            ones63 = persist.tile([NT, NT], FP32)
            nc.vector.memset(ones63[:], 1.0)
            ones1 = persist.tile([1, 128], FP32)
            nc.vector.memset(ones1[:], 1.0)
            ones128b = persist.tile([128, 1], BF16)
            nc.vector.memset(ones128b[:], 1.0)

            # ---- 1. one load: xf[t, rt*256 + c] = x_rt[128 t + c] ----
            xf = bigp.tile([NT, RT * HALO], FP32)
            ld = nc.sync.dma_start(
                xf[:], bass.AP(xin, 0, [[128, NT], [L, RT], [1, HALO]]))

            # ---- 2. batched stats (means of x[0:com] per rt) ----
            xa3 = xf[:].rearrange("p (r c) -> p r c", r=RT)[:, :, 0:128]
            rowsum = spool.tile([NT, RT], FP32, tag="rowsum")
            red = nc.vector.tensor_reduce(
                rowsum[:], xa3, mybir.AxisListType.X, mybir.AluOpType.add)
            add_dep_helper(red.ins, ld.ins, reason="reduce reads xf (3d view)")
            bcast = sps.tile([NT, RT], FP32, tag="bcast")
            nc.tensor.matmul(bcast[:], ones63[:], rowsum[:],
                             start=True, stop=True)
            m0 = spool.tile([NT, RT], FP32, tag="m0")
            nc.scalar.mul(m0[:], bcast[:], 1.0 / COM)

            # ---- 3. weights (centered, bf16) + raw stream (bf16) ----
            wt = bigp.tile([NT, RT * 128], BF16)
            for rt in range(RT):
                nc.vector.tensor_scalar_sub(
                    wt[:, rt * 128:(rt + 1) * 128],
                    xf[:, rt * HALO:rt * HALO + 128],
                    m0[:, rt:rt + 1])
            xs = bigp.tile([NT, RT * HALO], BF16)
            half = RT * HALO // 2
            nc.scalar.copy(xs[:, 0:half], xf[:, 0:half])
            nc.vector.tensor_copy(xs[:, half:], xf[:, half:])

            # ---- 4. H matmuls + PSUM->SBUF (bf16 cast) ----
            h_all = bigp.tile([128, RT * HALO], BF16)
            for rt in range(RT):
                h_ps = hps.tile([128, HALO], FP32, tag="h")
                nc.tensor.matmul(
                    h_ps[:], wt[:, rt * 128:(rt + 1) * 128],
                    xs[:, rt * HALO:(rt + 1) * HALO], start=True, stop=True)
                cp = h_all[:, rt * HALO:(rt + 1) * HALO]
                if rt % 2 == 0:
                    nc.vector.tensor_copy(cp, h_ps[:])
                else:
                    nc.scalar.copy(cp, h_ps[:])

            # ---- 5. deskew via DRAM bounce (bf16), pipelined halves ----
            hd = hdp.tile([128, RT * HALO], BF16)
            rbig = bigp.tile([128, RT * NK], BF16)
            CW = RT * HALO  # 2048 row pitch in hd
            for lo, hi in ((0, 64), (64, 128)):
                w = nc.sync.dma_start(hd[lo:hi, :], h_all[lo:hi, :])
                diag = bass.AP(hd[:].tensor, lo * (CW + 1),
                               [[CW + 1, hi - lo], [HALO, RT], [1, NK]])
                r = nc.scalar.dma_start(rbig[lo:hi, :], diag)
                add_dep_helper(r.ins, w.ins, reason="deskew reads hd")

            # ---- 6. c[1..128] per rt -> cps columns; c0 -> [1, RT] ----
            cps = sps.tile([128, RT], FP32, tag="cps")
            for rt in range(RT):
                nc.tensor.matmul(
                    cps[:, rt:rt + 1],
                    rbig[:, rt * NK + 1:(rt + 1) * NK],
                    ones128b[:], start=True, stop=True)
            c0rhs = rbig[:].rearrange("p (r k) -> p r k", r=RT)[:, :, 0:1]
            c0ps = sps.tile([1, RT], FP32, tag="c0ps")
            nc.tensor.matmul(c0ps[:], ones128b[:], c0rhs.squeeze(2),
                             start=True, stop=True)

            # ---- 7. normalize + L1 partials ----
            cs = spool.tile([128, RT], FP32, tag="cs")
            nc.vector.tensor_copy(cs[:], cps[:])
            c0s = spool.tile([1, RT], FP32, tag="c0s")
            nc.scalar.copy(c0s[:], c0ps[:])
            rec = spool.tile([1, RT], FP32, tag="rec")
            nc.vector.reciprocal(rec[:], c0s[:])
            rps = sps.tile([128, RT], FP32, tag="rps")
            nc.tensor.matmul(rps[:], ones1[:], rec[:], start=True, stop=True)
            rbs = spool.tile([128, RT], FP32, tag="rbs")
            nc.scalar.copy(rbs[:], rps[:])

            hf = RT // 2
            t1 = spool.tile([128, hf], FP32, tag="t1")
            nc.vector.tensor_mul(t1[:], cs[:, 0:hf], rbs[:, 0:hf])
            t2 = spool.tile([128, hf], FP32, tag="t2")
            nc.vector.tensor_mul(t2[:], cs[:, hf:RT], rbs[:, hf:RT])
            d = spool.tile([128, hf], FP32, tag="d")
            nc.vector.tensor_sub(d[:], t1[:], t2[:])
            absr = spool.tile([128, 1], FP32, tag="absr")
            nc.vector.tensor_reduce(
                absr[:], d[:], mybir.AxisListType.X, mybir.AluOpType.add,
                apply_absolute_value=True)
            nc.sync.dma_start(out[:], absr[:])

    nc.compile()
    return nc


_CACHE = {}


def _get_program():
    if "nc" not in _CACHE:
        _CACHE["nc"] = build_program()
    return _CACHE["nc"]


def make_in_maps(fake: np.ndarray, real: np.ndarray):
    fake = np.asarray(fake, dtype=np.float32).reshape(B, L)
    real = np.asarray(real, dtype=np.float32).reshape(B, L)
    in_maps = []
    for c in range(N_CORES):
        rows = slice(c * ROWS_PER_CORE, (c + 1) * ROWS_PER_CORE)
        xin = np.concatenate([fake[rows], real[rows]], axis=0)
        in_maps.append({"xin": np.ascontiguousarray(xin)})
    return in_maps


def run(in_maps, **kwargs):
    """Run the SPMD program; returns (loss, BassKernelResults)."""
    res = run_bass_kernel_spmd(
        _get_program(), in_maps, list(range(N_CORES)), **kwargs
    )
    total = np.float64(0.0)
    for c in range(N_CORES):
        total += np.asarray(res.results[c]["out"], dtype=np.float64).sum()
    return np.float32(total / (B * NK)), res


def kernel(fake: np.ndarray, real: np.ndarray) -> np.ndarray:
    loss, _ = run(make_in_maps(fake, real))
    return loss


# revision 6
# speedup vs baseline: 1.8727x; 1.2785x over previous
"""AutoCorrelationLoss Trainium2 kernel (8-core SPMD, data-parallel over batch).

Math: for each row x (length L=8192), with com = L - 128 = 8064 = 63*128:
  ac[k] = mean(x0c * (Y_k - mean(Y_k)))  where x0c = x[:com] - mean(x[:com])
Since sum(x0c) = 0 the mean(Y_k) term vanishes:
  com * ac[k] = c[k] = sum_j x0c[j] * x[j+k]
Decompose j = 128*t + p (t<63, p<128), T[t, f] = x[128t + f] (f<256),
Tc = T[:, :128] - mean(x[:com]).  With H = Tc.T @ T  ([128, 256]):
  c[k] = sum_j H[j, j+k]   (a skew-diagonal sum, k = 0..128)
r[k] = c[k]/c[0]; loss = mean_{b,k} |r_fake - r_real|.  r[0] == 1 on both
sides so the k=0 term contributes 0 — only c[1..128] plus the divisor c[0]
are needed.

Per core: 8 row-tensors (rt = xin row; rows 0-3 fake, 4-7 real).  Layout
packs rt pairs onto partition halves: xin row 2i lives on partitions 0:63
(chunk t = partition), row 2i+1 on partitions 64:127.  This engages both
8-port DMA halves on the load and lets each H-matmul pair run concurrently
in separate PE row-groups (tile_position (0,0) / (64,0)).

Pipeline:
  1. two parallel strided loads (sync + scalar HWDGE rings) -> xf fp32
  2. stats: two 3D reduces, one block-diagonal ones-matmul broadcast,
     negated means; centering fused with fp32->bf16 cast (DVE+ACT)
  3. 8 bf16 matmuls as 4 concurrent row-group pairs -> H in PSUM
  4. PSUM->SBUF copies write H bf16 interleave-4 per 4-row group, so each
     deskew group bounce is: one contiguous 256KB write (128 x 2KB
     descriptors) + one diagonal re-read R[j, k*4+u] = H[j,(j+k)*4+u]
     that is a single 128 x 1032B-descriptor DMA.  Two groups pipeline
     across the two HWDGE rings.
  5. per rt a transposed matmul (lhsT = stride-4 slice of R, rhs = ones)
     -> c[1..128] as a [128, 8] column block; ones-matmuls give c0 [1, 8]
  6. reciprocal of c0, broadcast via rank-1 matmul, r = c * (1/c0),
     |r_f - r_r| reduced on-chip to a single scalar (ones-matmul across
     partitions) -> out [1, 1] (single-descriptor DMA)
Host sums 8 cores' scalars and divides by B*(NCOEF+1).
"""

import os
import sys

sys.path.insert(0, "/opt/trn_rl_repo")

import numpy as np

import concourse.bacc as bacc
import concourse.bass as bass
import concourse.mybir as mybir
import concourse.tile as tile
from concourse.bass_utils import run_bass_kernel_spmd
from concourse.tile_rust import add_dep_helper

B, L = 32, 8192
NCOEF = 128            # lags 0..128 -> 129 values
COM = L - NCOEF        # 8064 = 63 * 128
NT = 63                # contraction chunks
HALO = 256             # halo width per chunk
NK = NCOEF + 1         # 129
N_CORES = 8
ROWS_PER_CORE = B // N_CORES      # 4 batch rows per core
RT = 2 * ROWS_PER_CORE            # 8 row-tensors (fake rows then real rows)
NPAIR = RT // 2                   # 4 even/odd partition-packed pairs
GW = 4 * HALO                     # deskew group width (4 rts interleaved)

FP32 = mybir.dt.float32
BF16 = mybir.dt.bfloat16


def build_program():
    nc = bacc.Bacc(
        "TRN2",
        target_bir_lowering=False,
        debug=False,
        num_devices=N_CORES,
    )

    xin = nc.dram_tensor("xin", (RT, L), FP32, kind="ExternalInput")
    out = nc.dram_tensor("out", (1, 1), FP32, kind="ExternalOutput")

    with tile.TileContext(nc) as tc:
        with (
            tc.tile_pool(name="persist", bufs=1) as persist,
            tc.tile_pool(name="big", bufs=1) as bigp,
            tc.tile_pool(name="spool", bufs=4) as spool,
            tc.tile_pool(name="hdp", bufs=1, space=bass.MemorySpace.DRAM) as hdp,
            tc.tile_pool(name="hps", bufs=3, space=bass.MemorySpace.PSUM) as hps,
            tc.tile_pool(name="sps", bufs=1, space=bass.MemorySpace.PSUM) as sps,
        ):
            ones1b = persist.tile([1, 128], BF16)
            nc.gpsimd.memset(ones1b[:], 1.0)
            ones128b = persist.tile([128, 1], BF16)
            nc.gpsimd.memset(ones128b[:], 1.0)
            ones128f = persist.tile([128, 1], FP32)
            nc.gpsimd.memset(ones128f[:], 1.0)
            # block-diagonal ones: sums partitions 0:63 / 64:127 separately
            w2 = persist.tile([128, 128], FP32)
            nc.gpsimd.memset(w2[:], 0.0)
            nc.gpsimd.memset(w2[0:NT, 0:NT], 1.0)
            nc.gpsimd.memset(w2[64:64 + NT, 64:64 + NT], 1.0)

            # ---- 1. loads: xf[64e+t, i*256+c] = xin[2i+e, 128t+c] ----
            xf = bigp.tile([128, NPAIR * HALO], FP32)
            ld_e = nc.sync.dma_start(
                xf[0:NT, :],
                bass.AP(xin, 0, [[128, NT], [2 * L, NPAIR], [1, HALO]]))
            ld_o = nc.scalar.dma_start(
                xf[64:64 + NT, :],
                bass.AP(xin, L, [[128, NT], [2 * L, NPAIR], [1, HALO]]))

            # ---- 2. stats -> negated means [128, 4] ----
            rowsum = spool.tile([128, NPAIR], FP32, tag="rowsum")
            nc.gpsimd.memset(rowsum[:], 0.0)
            for lo, ld in ((0, ld_e), (64, ld_o)):
                view = xf[lo:lo + NT, :].rearrange(
                    "p (r c) -> p r c", r=NPAIR)[:, :, 0:128]
                red = nc.vector.tensor_reduce(
                    rowsum[lo:lo + NT, :], view,
                    mybir.AxisListType.X, mybir.AluOpType.add)
                add_dep_helper(red.ins, ld.ins, reason="reduce reads xf view")
            bcast = sps.tile([128, NPAIR], FP32, tag="bcast")
            nc.tensor.matmul(bcast[:], w2[:], rowsum[:], start=True, stop=True)
            negm = spool.tile([128, NPAIR], FP32, tag="negm")
            nc.scalar.mul(negm[:], bcast[:], -1.0 / COM)

            # ---- 3. weights (centered bf16) + raw stream (bf16) ----
            wt = bigp.tile([128, NPAIR * 128], BF16)
            for i in range(NPAIR):
                dst = wt[:, i * 128:(i + 1) * 128]
                src = xf[:, i * HALO:i * HALO + 128]
                sc = negm[:, i:i + 1]
                if i % 2 == 0:
                    nc.vector.tensor_scalar_add(dst, src, sc)
                else:
                    nc.scalar.add(dst, src, sc)
            xs = bigp.tile([128, NPAIR * HALO], BF16)
            cv = nc.vector.tensor_copy(xs[0:NT, :], xf[0:NT, :])
            add_dep_helper(cv.ins, ld_e.ins, reason="cast reads xf even")
            cs_ = nc.scalar.copy(xs[64:64 + NT, :], xf[64:64 + NT, :])
            add_dep_helper(cs_.ins, ld_o.ins, reason="cast reads xf odd")

            # ---- 4/5. per deskew group: H matmuls, bounce, c matmuls ----
            cps = sps.tile([128, RT], FP32, tag="cps")
            c0ps = sps.tile([1, RT], FP32, tag="c0ps")
            wr_eng = {0: nc.sync, 1: nc.sync}
            rd_eng = {0: nc.scalar, 1: nc.scalar}
            for g in range(2):
                h_all = bigp.tile([128, GW], BF16, tag=f"hall{g}")
                hv = h_all[:].rearrange("p (m u) -> p m u", u=4)
                copies = []
                for pi in (2 * g, 2 * g + 1):  # pair index
                    for e, (plo, tp) in enumerate(((0, (0, 0)), (64, (64, 0)))):
                        h_ps = hps.tile([128, HALO], FP32, tag="h")
                        nc.tensor.matmul(
                            h_ps[:],
                            wt[plo:plo + NT, pi * 128:(pi + 1) * 128],
                            xs[plo:plo + NT, pi * HALO:(pi + 1) * HALO],
                            start=True, stop=True, tile_position=tp)
                        u = 2 * (pi - 2 * g) + e   # rt = 4g + u = xin row
                        cp = hv[:, :, u]
                        if u % 2 == 0:
                            copies.append(nc.vector.tensor_copy(cp, h_ps[:]))
                        else:
                            copies.append(nc.scalar.copy(cp, h_ps[:]))

                hd = hdp.tile([128, GW], BF16, tag=f"hd{g}")
                w = wr_eng[g].dma_start(hd[:], h_all[:])
                for cp_i in copies:
                    add_dep_helper(w.ins, cp_i.ins,
                                   reason="bounce write reads h_all (views)")
                rbig = bigp.tile([128, 4 * NK], BF16, tag=f"rbig{g}")
                diag = bass.AP(hd[:].tensor, 0, [[GW + 4, 128], [1, 4 * NK]])
                r = rd_eng[g].dma_start(rbig[:], diag)
                add_dep_helper(r.ins, w.ins, reason="deskew reads hd")

                rbv = rbig[:].rearrange("p (k u) -> p k u", u=4)
                for u in range(4):
                    mm = nc.tensor.matmul(
                        cps[:, 4 * g + u:4 * g + u + 1],
                        rbv[:, 1:NK, u], ones128b[:],
                        start=True, stop=True)
                    add_dep_helper(mm.ins, r.ins, reason="c mm reads rbig")
                mm0 = nc.tensor.matmul(
                    c0ps[:, 4 * g:4 * g + 4], ones128b[:], rbig[:, 0:4],
                    start=True, stop=True)
                add_dep_helper(mm0.ins, r.ins, reason="c0 mm reads rbig")

            # ---- 6. normalize + L1 -> single scalar ----
            cs = spool.tile([128, RT], FP32, tag="cs")
            nc.vector.tensor_copy(cs[:], cps[:])
            rec = spool.tile([1, RT], BF16, tag="rec")
            with nc.allow_low_precision("bf16 1/c0 feeds a bf16 matmul; "
                                        "loss tolerance is 2e-2"):
                nc.vector.reciprocal(rec[:], c0ps[:])
            rps = sps.tile([128, RT], FP32, tag="rps")
            nc.tensor.matmul(rps[:], ones1b[:], rec[:], start=True, stop=True)
            rbs = spool.tile([128, RT], FP32, tag="rbs")
            nc.scalar.copy(rbs[:], rps[:])

            hf = RT // 2
            t1 = spool.tile([128, hf], FP32, tag="t1")
            nc.vector.tensor_mul(t1[:], cs[:, 0:hf], rbs[:, 0:hf])
            t2 = spool.tile([128, hf], FP32, tag="t2")
            nc.vector.tensor_mul(t2[:], cs[:, hf:RT], rbs[:, hf:RT])
            d = spool.tile([128, hf], FP32, tag="d")
            nc.vector.tensor_sub(d[:], t1[:], t2[:])
            absr = spool.tile([128, 1], FP32, tag="absr")
            nc.vector.tensor_reduce(
                absr[:], d[:], mybir.AxisListType.X, mybir.AluOpType.add,
                apply_absolute_value=True)
            tps = sps.tile([1, 1], FP32, tag="tps")
            nc.tensor.matmul(tps[:], absr[:], ones128f[:],
                             start=True, stop=True)
            ts_sb = spool.tile([1, 1], FP32, tag="ts")
            nc.scalar.copy(ts_sb[:], tps[:])
            nc.sync.dma_start(out[:], ts_sb[:], single_packet=True)

    nc.compile()
    return nc


_CACHE = {}


def _get_program():
    if "nc" not in _CACHE:
        _CACHE["nc"] = build_program()
    return _CACHE["nc"]


def make_in_maps(fake: np.ndarray, real: np.ndarray):
    fake = np.asarray(fake, dtype=np.float32).reshape(B, L)
    real = np.asarray(real, dtype=np.float32).reshape(B, L)
    in_maps = []
    for c in range(N_CORES):
        rows = slice(c * ROWS_PER_CORE, (c + 1) * ROWS_PER_CORE)
        xin = np.concatenate([fake[rows], real[rows]], axis=0)
        in_maps.append({"xin": np.ascontiguousarray(xin)})
    return in_maps


def run(in_maps, **kwargs):
    """Run the SPMD program; returns (loss, BassKernelResults)."""
    res = run_bass_kernel_spmd(
        _get_program(), in_maps, list(range(N_CORES)), **kwargs
    )
    total = np.float64(0.0)
    for c in range(N_CORES):
        total += np.asarray(res.results[c]["out"], dtype=np.float64).sum()
    return np.float32(total / (B * NK)), res


def kernel(fake: np.ndarray, real: np.ndarray) -> np.ndarray:
    loss, _ = run(make_in_maps(fake, real))
    return loss
